# revision 25
# baseline (speedup 1.0000x reference)
"""Trainium2 Bass kernel for nn_CartesianProductClassifier.

out[b,i,j] = sigmoid(MLP(concat(x[b,j], x[b,i])))  for x [8, 512, 32].

Math restructuring:
  layer1: h1[b,i,j] = relu(x[b,j]@W1[:32] + x[b,i]@W1[32:] + b1)
          = relu(A[b,j] + C[b,i])   with A = x@W1_top (N-sized),
            C = x@W1_bot + b1 (N-sized)  -> first layer is O(N), not O(N^2).
  layers 2-4 run on the N^2 grid with 2 batches packed block-diagonally
  on the 128 partitions; matmuls use f32r (full PE rate for 4-byte data),
  except layer 3 which uses bf16 so its second matmul may write PSUM
  partition base 64 (f32r matmuls must write base 0).  Layer 4 uses
  column-shifted W4 matrices accumulating 64 output rows into one PSUM
  bank so a single sigmoid covers them.

Sharding: core c handles rows i in [64c, 64c+64) of all 8 batches.
"""

import numpy as np

B, N, D = 8, 512, 32
NCORES = 8
RPC = N // NCORES  # rows per core = 64

_PROG = None


def _build_program():
    import concourse.mybir as mybir
    import concourse.tile as tile
    from concourse import bacc
    from concourse.tile import add_dep_helper

    dt = mybir.dt
    F32 = dt.float32
    F32R = dt.float32r
    BF16 = dt.bfloat16
    AF = mybir.ActivationFunctionType
    OP = mybir.AluOpType

    nc = bacc.Bacc(
        "TRN2", target_bir_lowering=False, debug=False, num_devices=NCORES
    )

    # xT2: [64, 4*512] col = bp*512 + j; rows 0:32 features of batch 2bp,
    #      rows 32:64 of batch 2bp+1.   xcT2: same layout, col = bp*64 + i.
    xT2 = nc.dram_tensor("xT2", [2 * D, 4 * N], BF16, kind="ExternalInput")
    # wcomb packs [xcT2 (256) | w1tbd (128) | w1bbd (128)] on 64 partitions
    wcomb = nc.dram_tensor("wcomb", [2 * D, 512], BF16, kind="ExternalInput")
    # w23 packs [w2bd (128) | w3bd (64)] on 128 partitions
    w23 = nc.dram_tensor("w23", [128, 192], BF16, kind="ExternalInput")
    w4sh = nc.dram_tensor("w4sh", [128, 32 * 128], BF16, kind="ExternalInput")
    # bcomb packs [b1s | b2s | b3s | b4s] columns
    bcomb = nc.dram_tensor("bcomb", [128, 4], F32, kind="ExternalInput")
    out = nc.dram_tensor("out", [B, RPC, N], F32, kind="ExternalOutput")

    with tile.TileContext(nc) as tc:
        with (
            tc.tile_pool(name="const", bufs=1) as const,
            tc.tile_pool(name="h1p", bufs=10) as h1p,
            tc.tile_pool(name="h2p", bufs=8) as h2p,
            tc.tile_pool(name="h3p", bufs=8) as h3p,
            tc.tile_pool(name="sigp", bufs=2) as sigp,
            tc.tile_pool(name="psA", bufs=2, space="PSUM") as psA,
            tc.tile_pool(name="psB", bufs=3, space="PSUM") as psB,
            tc.tile_pool(name="psC", bufs=1, space="PSUM") as psC,
        ):
            # ---------- constant loads ----------
            # each dma_start costs ~600ns of issue time on the sync queue, so
            # order by what gates the pipeline: wcomb (stage A weights + xc),
            # bcomb (biases), xT2, w23
            # dummy ACT op first: forces the activation-table load to happen
            # during the DMA issue phase instead of on the first real copy
            scratch = const.tile([128, 1], F32, tag="scratch")
            nc.vector.memset(scratch[:], 0.0)
            nc.scalar.activation(scratch[:], scratch[:], AF.Relu, scale=1.0)
            wcomb_sb = const.tile([2 * D, 512], BF16, tag="wcomb")
            nc.sync.dma_start(wcomb_sb[:], wcomb[:])
            xT2_sb = const.tile([2 * D, 4 * N], BF16, tag="xT2")
            nc.sync.dma_start(xT2_sb[:, 0:N], xT2[:, 0:N])
            bcomb_sb = const.tile([128, 4], F32, tag="bcomb")
            nc.sync.dma_start(bcomb_sb[:], bcomb[:])
            w23_sb = const.tile([128, 192], BF16, tag="w23")
            nc.sync.dma_start(w23_sb[:], w23[:])
            nc.sync.dma_start(xT2_sb[:, N:], xT2[:, N:])
            xcT2_sb = wcomb_sb[:, 0:256]
            w1tbd_sb = wcomb_sb[:, 256:384]
            w1bbd_sb = wcomb_sb[:, 384:512]
            w2bd_sb = w23_sb[:, 0:128]
            w3bd_sb = w23_sb[:, 128:192]
            b1s_sb = bcomb_sb[:, 0:1]
            b2s_sb = bcomb_sb[:, 1:2]
            b3s_sb = bcomb_sb[:, 2:3]
            b4s_sb = bcomb_sb[:, 3:4]
            w4sh_sb = const.tile([128, 32 * 128], BF16, tag="w4sh")
            # one 256KB chunk per queue, in parallel; chunk q only gates
            # mm4 twins 8q..8q+7
            for q in range(4):
                nc.gpsimd.dma_start(
                    w4sh_sb[:, q * 1024 : (q + 1) * 1024],
                    w4sh[:, q * 1024 : (q + 1) * 1024],
                )

            # ---------- stage A: AT2 [128, 2048], Cpp [128, 256] ----------
            # AT2[:, bp*512 + j]: p<64 -> A-hidden of batch 2bp at col j,
            #                     p>=64 -> batch 2bp+1.
            AT2 = const.tile([128, 4 * N], BF16, tag="AT2")
            Cpp = const.tile([128, 4 * RPC], F32, tag="Cpp")
            # Cpp's matmul first: it only needs wcomb (xT2 still in flight)
            psc = psA.tile([128, N], F32, tag="ps2", name="psc")
            nc.tensor.matmul(
                psc[:, 0 : 4 * RPC],
                w1bbd_sb,
                xcT2_sb,
                start=True,
                stop=True,
            )
            # Cpp = psc + b1 (no relu here; relu happens after adding A)
            nc.vector.tensor_scalar(
                Cpp[:], psc[:, 0 : 4 * RPC], b1s_sb, None, OP.add
            )
            psa0 = psA.tile([128, N], F32, tag="ps2", name="psa0")
            nc.tensor.matmul(
                psa0[:], w1tbd_sb, xT2_sb[:, 0:N], start=True, stop=True
            )
            nc.vector.tensor_copy(AT2[:, 0:N], psa0[:])
            for bp in range(1, 4):
                psa = psA.tile([128, N], F32, tag="ps2", name=f"psa{bp}")
                nc.tensor.matmul(
                    psa[:],
                    w1tbd_sb,
                    xT2_sb[:, bp * N : (bp + 1) * N],
                    start=True,
                    stop=True,
                )
                nc.scalar.activation(
                    AT2[:, bp * N : (bp + 1) * N], psa[:], AF.Copy, scale=1.0
                )

            # ---------- main loop: software-pipelined 2-twin pairs ----------
            # 64 global pairs (bp = pair // 16, t in {2u, 2u+1}).  Per pair
            # cycle we emit: h1 x4 + mm2 x4 (pair p), relu2 x2 (pair p),
            # mm3 x4 + relu3 x2 (pair p-1), mm4 x2 (pair p-2).  The 2-stage
            # skew keeps the PE stream free of same-cycle waits and halves
            # weight-set reloads (w2 once per pair).
            sig_tiles = {}

            def emit_h1(tw):
                bp, t = tw
                out_t = []
                for k, ii in enumerate((2 * t, 2 * t + 1)):
                    h1 = h1p.tile(
                        [128, N], BF16, tag="h1", name=f"h1_{bp}_{t}_{k}"
                    )
                    nc.vector.tensor_scalar(
                        h1[:],
                        AT2[:, bp * N : (bp + 1) * N],
                        Cpp[:, bp * RPC + ii : bp * RPC + ii + 1],
                        0.0,
                        OP.add,
                        OP.max,
                    )
                    out_t.append(h1)
                return out_t

            def emit_mm2(tw, h1s):
                bp, t = tw
                ps2d = psA.tile(
                    [128, 2 * N], F32, tag="ps2", name=f"ps2_{bp}_{t}"
                )
                nc.tensor.matmul(
                    ps2d[:, 0:N], w2bd_sb, h1s[0][:], start=True, stop=True
                )
                nc.tensor.matmul(
                    ps2d[:, N : 2 * N],
                    w2bd_sb,
                    h1s[1][:],
                    start=True,
                    stop=True,
                    skip_group_check=True,
                )
                return ps2d

            def emit_relu2(tw, ps2d):
                bp, t = tw
                h2ab = h2p.tile([128, 2 * N], BF16, tag="h2", name=f"h2_{bp}_{t}")
                nc.scalar.activation(
                    h2ab[:], ps2d[:], AF.Relu, bias=b2s_sb, scale=1.0
                )
                return h2ab

            def emit_mm3(tw, h2ab):
                bp, t = tw
                ps3 = psB.tile([128, N], F32, tag="ps3", name=f"ps3_{bp}_{t}")
                nc.tensor.matmul(
                    ps3[0:64, :],
                    w3bd_sb,
                    h2ab[:, 0:N],
                    start=True,
                    stop=True,
                    skip_group_check=True,
                )
                nc.tensor.matmul(
                    ps3[64:128, :],
                    w3bd_sb,
                    h2ab[:, N : 2 * N],
                    start=True,
                    stop=True,
                    skip_group_check=True,
                )
                return ps3

            def emit_relu3(tw, ps3):
                bp, t = tw
                h3 = h3p.tile([128, N], BF16, tag="h3", name=f"h3_{bp}_{t}")
                # every 12th twin's relu3 runs on the scalar engine to
                # balance DVE (h1+relu3) against ACT (relu2)
                if (bp * 32 + t) % 10 == 5:
                    nc.scalar.activation(
                        h3[:], ps3[:], AF.Relu, bias=b3s_sb, scale=1.0
                    )
                else:
                    nc.vector.tensor_scalar(
                        h3[:], ps3[:], b3s_sb, 0.0, OP.add, OP.max
                    )
                return h3

            def emit_mm4(tw, h3):
                bp, t = tw
                if bp not in sig_tiles:
                    sig_tiles[bp] = psC.tile(
                        [128, N], F32, tag="sig", name=f"sig{bp}"
                    )
                nc.tensor.matmul(
                    sig_tiles[bp][:],
                    w4sh_sb[:, t * 128 : (t + 1) * 128],
                    h3[:],
                    start=(t == 0),
                    stop=(t == 31),
                    skip_group_check=True,
                )
                if t == 31:
                    sig_ps = sig_tiles.pop(bp)
                    sig_sb = sigp.tile(
                        [128, N], F32, tag="sig_sb", name=f"sigsb{bp}"
                    )
                    # split the sigmoid by column half so the out-DMAs start
                    # earlier; spread DMAs across both queues so the final
                    # drain overlaps
                    for h in range(2):
                        cs = slice(h * (N // 2), (h + 1) * (N // 2))
                        nc.scalar.activation(
                            sig_sb[:, cs],
                            sig_ps[:, cs],
                            AF.Sigmoid,
                            bias=b4s_sb,
                            scale=1.0,
                        )
                        nc.sync.dma_start(out[2 * bp, :, cs], sig_sb[0:64, cs])
                        nc.gpsimd.dma_start(
                            out[2 * bp + 1, :, cs], sig_sb[64:128, cs]
                        )

            NP = 64  # global pairs
            state = {}  # pair -> dict of stage products
            for p in range(NP + 2):
                if p < NP:
                    tws = [(p // 16, 2 * (p % 16)), (p // 16, 2 * (p % 16) + 1)]
                    st = {"tws": tws}
                    st["h1"] = [emit_h1(tw) for tw in tws]
                    st["ps2"] = [
                        emit_mm2(tw, h1s) for tw, h1s in zip(tws, st["h1"])
                    ]
                    st["h2"] = [
                        emit_relu2(tw, ps2d)
                        for tw, ps2d in zip(tws, st["ps2"])
                    ]
                    state[p] = st
                if 0 <= p - 1 < NP:
                    st = state[p - 1]
                    st["ps3"] = [
                        emit_mm3(tw, h2ab)
                        for tw, h2ab in zip(st["tws"], st["h2"])
                    ]
                    st["h3"] = [
                        emit_relu3(tw, ps3)
                        for tw, ps3 in zip(st["tws"], st["ps3"])
                    ]
                if 0 <= p - 2 < NP:
                    st = state.pop(p - 2)
                    for tw, h3 in zip(st["tws"], st["h3"]):
                        emit_mm4(tw, h3)

    nc.compile()
    return nc


def _get_program():
    global _PROG
    if _PROG is None:
        _PROG = _build_program()
    return _PROG


def prep_in_maps(inputs):
    import ml_dtypes

    x = np.ascontiguousarray(np.asarray(inputs["x"], dtype=np.float32))
    W1 = np.asarray(inputs["W1"], dtype=np.float32)
    b1 = np.asarray(inputs["b1"], dtype=np.float32)
    W2 = np.asarray(inputs["W2"], dtype=np.float32)
    b2 = np.asarray(inputs["b2"], dtype=np.float32)
    W3 = np.asarray(inputs["W3"], dtype=np.float32)
    b3 = np.asarray(inputs["b3"], dtype=np.float32)
    W4 = np.asarray(inputs["W4"], dtype=np.float32)
    b4 = np.asarray(inputs["b4"], dtype=np.float32)

    bf16 = ml_dtypes.bfloat16
    w1tbd = np.zeros((2 * D, 128), bf16)
    w1tbd[:D, :64] = W1[:D].astype(bf16)
    w1tbd[D:, 64:] = W1[:D].astype(bf16)
    w1bbd = np.zeros((2 * D, 128), bf16)
    w1bbd[:D, :64] = W1[D:].astype(bf16)
    w1bbd[D:, 64:] = W1[D:].astype(bf16)
    w23 = np.zeros((128, 192), bf16)
    w23[:64, 0:64] = W2.astype(bf16)
    w23[64:, 64:128] = W2.astype(bf16)
    w23[:64, 128:160] = W3.astype(bf16)
    w23[64:, 160:192] = W3.astype(bf16)
    # w4sh[t]: rows 32r..32r+32 (r = h3 row-block) -> output column
    # 64*(r&1) + 2t + (r>>1)
    w4sh = np.zeros((128, 32 * 128), bf16)
    w4c = W4[:, 0].astype(bf16)
    for t in range(32):
        for rr in range(4):
            col = t * 128 + 64 * (rr & 1) + 2 * t + (rr >> 1)
            w4sh[32 * rr : 32 * rr + 32, col] = w4c

    # xT2 [64, 4*512]: col = bp*512 + j, rows 0:32 batch 2bp, 32:64 batch 2bp+1
    xT2 = np.zeros((2 * D, 4 * N), bf16)
    for bp in range(4):
        xT2[:D, bp * N : (bp + 1) * N] = x[2 * bp].T.astype(bf16)
        xT2[D:, bp * N : (bp + 1) * N] = x[2 * bp + 1].T.astype(bf16)

    bcomb = np.zeros((128, 4), np.float32)
    bcomb[:, 0] = np.concatenate([b1, b1])
    bcomb[:, 1] = np.concatenate([b2, b2])
    bcomb[:, 2] = np.tile(b3, 4)
    bcomb[:, 3] = b4[0]

    common = {
        "xT2": xT2,
        "w23": w23,
        "w4sh": w4sh,
        "bcomb": bcomb,
    }
    in_maps = []
    for c in range(NCORES):
        xc = x[:, c * RPC : (c + 1) * RPC, :]  # [8, 64, 32]
        wcomb = np.zeros((2 * D, 512), bf16)
        for bp in range(4):
            wcomb[:D, bp * RPC : (bp + 1) * RPC] = xc[2 * bp].T.astype(bf16)
            wcomb[D:, bp * RPC : (bp + 1) * RPC] = xc[2 * bp + 1].T.astype(bf16)
        wcomb[:, 256:384] = w1tbd
        wcomb[:, 384:512] = w1bbd
        in_maps.append({**common, "wcomb": wcomb})
    return in_maps


def assemble(per_core_outs):
    full = np.empty((B, N, N), np.float32)
    for c in range(NCORES):
        full[:, c * RPC : (c + 1) * RPC, :] = per_core_outs[c]
    return full


def run(inputs, trace=False):
    """Returns (full_output, BassKernelResults)."""
    from concourse.bass_utils import run_bass_kernel_spmd

    nc = _get_program()
    in_maps = prep_in_maps(inputs)
    res = run_bass_kernel_spmd(nc, in_maps, list(range(NCORES)), trace=trace)
    full = assemble([res.results[c]["out"] for c in range(NCORES)])
    return full, res


def kernel(**inputs):
    full, _ = run(inputs, trace=False)
    return full



# revision 30
# speedup vs baseline: 1.0157x; 1.0157x over previous
"""Trainium2 Bass kernel for nn_CartesianProductClassifier.

out[b,i,j] = sigmoid(MLP(concat(x[b,j], x[b,i])))  for x [8, 512, 32].

Math restructuring:
  layer1: h1[b,i,j] = relu(x[b,j]@W1[:32] + x[b,i]@W1[32:] + b1)
          = relu(A[b,j] + C[b,i])   with A = x@W1_top (N-sized),
            C = x@W1_bot + b1 (N-sized)  -> first layer is O(N), not O(N^2).
  layers 2-4 run on the N^2 grid with 2 batches packed block-diagonally
  on the 128 partitions; matmuls use f32r (full PE rate for 4-byte data),
  except layer 3 which uses bf16 so its second matmul may write PSUM
  partition base 64 (f32r matmuls must write base 0).  Layer 4 uses
  column-shifted W4 matrices accumulating 64 output rows into one PSUM
  bank so a single sigmoid covers them.

Sharding: core c handles rows i in [64c, 64c+64) of all 8 batches.
"""

import numpy as np

B, N, D = 8, 512, 32
NCORES = 8
RPC = N // NCORES  # rows per core = 64

_PROG = None


def _build_program():
    import concourse.mybir as mybir
    import concourse.tile as tile
    from concourse import bacc
    from concourse.tile import add_dep_helper

    dt = mybir.dt
    F32 = dt.float32
    F32R = dt.float32r
    BF16 = dt.bfloat16
    AF = mybir.ActivationFunctionType
    OP = mybir.AluOpType

    nc = bacc.Bacc(
        "TRN2", target_bir_lowering=False, debug=False, num_devices=NCORES
    )

    # xT2: [64, 4*512] col = bp*512 + j; rows 0:32 features of batch 2bp,
    #      rows 32:64 of batch 2bp+1.   xcT2: same layout, col = bp*64 + i.
    xT2 = nc.dram_tensor("xT2", [2 * D, 4 * N], BF16, kind="ExternalInput")
    # wcomb packs [xcT2 (256) | w1tbd (128) | w1bbd (128)] on 64 partitions
    wcomb = nc.dram_tensor("wcomb", [2 * D, 512], BF16, kind="ExternalInput")
    # w23 packs [w2bd (128) | w3bd (64)] on 128 partitions
    w23 = nc.dram_tensor("w23", [128, 192], BF16, kind="ExternalInput")
    # bcomb packs [b1s | b2s | b3s | b4s | w4c-tiled] columns
    bcomb = nc.dram_tensor("bcomb", [128, 5], F32, kind="ExternalInput")
    out = nc.dram_tensor("out", [B, RPC, N], F32, kind="ExternalOutput")

    with tile.TileContext(nc) as tc:
        with (
            tc.tile_pool(name="const", bufs=1) as const,
            tc.tile_pool(name="h1p", bufs=10) as h1p,
            tc.tile_pool(name="h2p", bufs=8) as h2p,
            tc.tile_pool(name="h3p", bufs=8) as h3p,
            tc.tile_pool(name="sigp", bufs=2) as sigp,
            tc.tile_pool(name="psA", bufs=2, space="PSUM") as psA,
            tc.tile_pool(name="psB", bufs=3, space="PSUM") as psB,
            tc.tile_pool(name="psC", bufs=1, space="PSUM") as psC,
        ):
            # ---------- constant loads ----------
            # each dma_start costs ~600ns of issue time on the sync queue, so
            # order by what gates the pipeline: wcomb (stage A weights + xc),
            # bcomb (biases), xT2, w23
            # dummy ACT op first: forces the activation-table load to happen
            # during the DMA issue phase instead of on the first real copy
            scratch = const.tile([128, 1], F32, tag="scratch")
            nc.vector.memset(scratch[:], 0.0)
            nc.scalar.activation(scratch[:], scratch[:], AF.Relu, scale=1.0)
            # w4sh is built on-chip (mostly zeros): zero it while DMAs fly
            w4sh_sb = const.tile([128, 32 * 128], BF16, tag="w4sh")
            nc.vector.memset(w4sh_sb[:], 0.0)
            # two DMA queues in parallel: sync gets wcomb/w23/xT2-rest,
            # scalar-queue gets the small bcomb + the urgent xT2 bp0 slice
            wcomb_sb = const.tile([2 * D, 512], BF16, tag="wcomb")
            nc.sync.dma_start(wcomb_sb[:], wcomb[:])
            bcomb_sb = const.tile([128, 5], F32, tag="bcomb")
            nc.scalar.dma_start(bcomb_sb[:], bcomb[:])
            xT2_sb = const.tile([2 * D, 4 * N], BF16, tag="xT2")
            nc.scalar.dma_start(xT2_sb[:, 0:N], xT2[:, 0:N])
            w23_sb = const.tile([128, 192], BF16, tag="w23")
            nc.sync.dma_start(w23_sb[:], w23[:])
            nc.sync.dma_start(xT2_sb[:, N:], xT2[:, N:])
            xcT2_sb = wcomb_sb[:, 0:256]
            w1tbd_sb = wcomb_sb[:, 256:384]
            w1bbd_sb = wcomb_sb[:, 384:512]
            w2bd_sb = w23_sb[:, 0:128]
            w3bd_sb = w23_sb[:, 128:192]
            b1s_sb = bcomb_sb[:, 0:1]
            b2s_sb = bcomb_sb[:, 1:2]
            b3s_sb = bcomb_sb[:, 2:3]
            b4s_sb = bcomb_sb[:, 3:4]
            # scatter w4 columns into the zeroed w4sh: block r has w4c at
            # rows 32r:32r+32, cols 64*(r&1) + (r>>1) + 130*t  (t = 0..31)
            for r in range(4):
                base = 64 * (r & 1) + (r >> 1)
                nc.vector.tensor_copy(
                    w4sh_sb[
                        32 * r : 32 * r + 32, base : base + 130 * 31 + 1 : 130
                    ],
                    bcomb_sb[32 * r : 32 * r + 32, 4:5].broadcast_to([32, 32]),
                )

            # ---------- stage A: AT2 [128, 2048], Cpp [128, 256] ----------
            # AT2[:, bp*512 + j]: p<64 -> A-hidden of batch 2bp at col j,
            #                     p>=64 -> batch 2bp+1.
            AT2 = const.tile([128, 4 * N], BF16, tag="AT2")
            Cpp = const.tile([128, 4 * RPC], F32, tag="Cpp")
            # Cpp's matmul first: it only needs wcomb (xT2 still in flight)
            psc = psA.tile([128, N], F32, tag="ps2", name="psc")
            nc.tensor.matmul(
                psc[:, 0 : 4 * RPC],
                w1bbd_sb,
                xcT2_sb,
                start=True,
                stop=True,
            )
            # Cpp = psc + b1 (no relu here; relu happens after adding A)
            nc.vector.tensor_scalar(
                Cpp[:], psc[:, 0 : 4 * RPC], b1s_sb, None, OP.add
            )
            psa0 = psA.tile([128, N], F32, tag="ps2", name="psa0")
            nc.tensor.matmul(
                psa0[:], w1tbd_sb, xT2_sb[:, 0:N], start=True, stop=True
            )
            nc.vector.tensor_copy(AT2[:, 0:N], psa0[:])
            for bp in range(1, 4):
                psa = psA.tile([128, N], F32, tag="ps2", name=f"psa{bp}")
                nc.tensor.matmul(
                    psa[:],
                    w1tbd_sb,
                    xT2_sb[:, bp * N : (bp + 1) * N],
                    start=True,
                    stop=True,
                )
                nc.scalar.activation(
                    AT2[:, bp * N : (bp + 1) * N], psa[:], AF.Copy, scale=1.0
                )

            # ---------- main loop: software-pipelined 2-twin pairs ----------
            # 64 global pairs (bp = pair // 16, t in {2u, 2u+1}).  Per pair
            # cycle we emit: h1 x4 + mm2 x4 (pair p), relu2 x2 (pair p),
            # mm3 x4 + relu3 x2 (pair p-1), mm4 x2 (pair p-2).  The 2-stage
            # skew keeps the PE stream free of same-cycle waits and halves
            # weight-set reloads (w2 once per pair).
            sig_tiles = {}

            def emit_h1(tw):
                bp, t = tw
                out_t = []
                for k, ii in enumerate((2 * t, 2 * t + 1)):
                    h1 = h1p.tile(
                        [128, N], BF16, tag="h1", name=f"h1_{bp}_{t}_{k}"
                    )
                    nc.vector.tensor_scalar(
                        h1[:],
                        AT2[:, bp * N : (bp + 1) * N],
                        Cpp[:, bp * RPC + ii : bp * RPC + ii + 1],
                        0.0,
                        OP.add,
                        OP.max,
                    )
                    out_t.append(h1)
                return out_t

            def emit_mm2(tw, h1s):
                bp, t = tw
                ps2d = psA.tile(
                    [128, 2 * N], F32, tag="ps2", name=f"ps2_{bp}_{t}"
                )
                nc.tensor.matmul(
                    ps2d[:, 0:N], w2bd_sb, h1s[0][:], start=True, stop=True
                )
                nc.tensor.matmul(
                    ps2d[:, N : 2 * N],
                    w2bd_sb,
                    h1s[1][:],
                    start=True,
                    stop=True,
                    skip_group_check=True,
                )
                return ps2d

            def emit_relu2(tw, ps2d):
                bp, t = tw
                h2ab = h2p.tile([128, 2 * N], BF16, tag="h2", name=f"h2_{bp}_{t}")
                nc.scalar.activation(
                    h2ab[:], ps2d[:], AF.Relu, bias=b2s_sb, scale=1.0
                )
                return h2ab

            def emit_mm3(tw, h2ab):
                bp, t = tw
                ps3 = psB.tile([128, N], F32, tag="ps3", name=f"ps3_{bp}_{t}")
                nc.tensor.matmul(
                    ps3[0:64, :],
                    w3bd_sb,
                    h2ab[:, 0:N],
                    start=True,
                    stop=True,
                    skip_group_check=True,
                )
                nc.tensor.matmul(
                    ps3[64:128, :],
                    w3bd_sb,
                    h2ab[:, N : 2 * N],
                    start=True,
                    stop=True,
                    skip_group_check=True,
                )
                return ps3

            def emit_relu3(tw, ps3):
                bp, t = tw
                h3 = h3p.tile([128, N], BF16, tag="h3", name=f"h3_{bp}_{t}")
                # every 12th twin's relu3 runs on the scalar engine to
                # balance DVE (h1+relu3) against ACT (relu2)
                if (bp * 32 + t) % 10 == 5:
                    nc.scalar.activation(
                        h3[:], ps3[:], AF.Relu, bias=b3s_sb, scale=1.0
                    )
                else:
                    nc.vector.tensor_scalar(
                        h3[:], ps3[:], b3s_sb, 0.0, OP.add, OP.max
                    )
                return h3

            def emit_mm4(tw, h3):
                bp, t = tw
                if bp not in sig_tiles:
                    sig_tiles[bp] = psC.tile(
                        [128, N], F32, tag="sig", name=f"sig{bp}"
                    )
                nc.tensor.matmul(
                    sig_tiles[bp][:],
                    w4sh_sb[:, t * 128 : (t + 1) * 128],
                    h3[:],
                    start=(t == 0),
                    stop=(t == 31),
                    skip_group_check=True,
                )
                if t == 31:
                    sig_ps = sig_tiles.pop(bp)
                    sig_sb = sigp.tile(
                        [128, N], F32, tag="sig_sb", name=f"sigsb{bp}"
                    )
                    # split the sigmoid by column half so the out-DMAs start
                    # earlier; spread DMAs across both queues so the final
                    # drain overlaps
                    for h in range(2):
                        cs = slice(h * (N // 2), (h + 1) * (N // 2))
                        nc.scalar.activation(
                            sig_sb[:, cs],
                            sig_ps[:, cs],
                            AF.Sigmoid,
                            bias=b4s_sb,
                            scale=1.0,
                        )
                        nc.sync.dma_start(out[2 * bp, :, cs], sig_sb[0:64, cs])
                        nc.gpsimd.dma_start(
                            out[2 * bp + 1, :, cs], sig_sb[64:128, cs]
                        )

            NP = 64  # global pairs
            state = {}  # pair -> dict of stage products
            for p in range(NP + 2):
                if p < NP:
                    tws = [(p // 16, 2 * (p % 16)), (p // 16, 2 * (p % 16) + 1)]
                    st = {"tws": tws}
                    st["h1"] = [emit_h1(tw) for tw in tws]
                    st["ps2"] = [
                        emit_mm2(tw, h1s) for tw, h1s in zip(tws, st["h1"])
                    ]
                    st["h2"] = [
                        emit_relu2(tw, ps2d)
                        for tw, ps2d in zip(tws, st["ps2"])
                    ]
                    state[p] = st
                if 0 <= p - 1 < NP:
                    st = state[p - 1]
                    st["ps3"] = [
                        emit_mm3(tw, h2ab)
                        for tw, h2ab in zip(st["tws"], st["h2"])
                    ]
                    st["h3"] = [
                        emit_relu3(tw, ps3)
                        for tw, ps3 in zip(st["tws"], st["ps3"])
                    ]
                if 0 <= p - 2 < NP:
                    st = state.pop(p - 2)
                    for tw, h3 in zip(st["tws"], st["h3"]):
                        emit_mm4(tw, h3)

    nc.compile()
    return nc


def _get_program():
    global _PROG
    if _PROG is None:
        _PROG = _build_program()
    return _PROG


def prep_in_maps(inputs):
    import ml_dtypes

    x = np.ascontiguousarray(np.asarray(inputs["x"], dtype=np.float32))
    W1 = np.asarray(inputs["W1"], dtype=np.float32)
    b1 = np.asarray(inputs["b1"], dtype=np.float32)
    W2 = np.asarray(inputs["W2"], dtype=np.float32)
    b2 = np.asarray(inputs["b2"], dtype=np.float32)
    W3 = np.asarray(inputs["W3"], dtype=np.float32)
    b3 = np.asarray(inputs["b3"], dtype=np.float32)
    W4 = np.asarray(inputs["W4"], dtype=np.float32)
    b4 = np.asarray(inputs["b4"], dtype=np.float32)

    bf16 = ml_dtypes.bfloat16
    w1tbd = np.zeros((2 * D, 128), bf16)
    w1tbd[:D, :64] = W1[:D].astype(bf16)
    w1tbd[D:, 64:] = W1[:D].astype(bf16)
    w1bbd = np.zeros((2 * D, 128), bf16)
    w1bbd[:D, :64] = W1[D:].astype(bf16)
    w1bbd[D:, 64:] = W1[D:].astype(bf16)
    w23 = np.zeros((128, 192), bf16)
    w23[:64, 0:64] = W2.astype(bf16)
    w23[64:, 64:128] = W2.astype(bf16)
    w23[:64, 128:160] = W3.astype(bf16)
    w23[64:, 160:192] = W3.astype(bf16)
    # xT2 [64, 4*512]: col = bp*512 + j, rows 0:32 batch 2bp, 32:64 batch 2bp+1
    xT2 = np.zeros((2 * D, 4 * N), bf16)
    for bp in range(4):
        xT2[:D, bp * N : (bp + 1) * N] = x[2 * bp].T.astype(bf16)
        xT2[D:, bp * N : (bp + 1) * N] = x[2 * bp + 1].T.astype(bf16)

    bcomb = np.zeros((128, 5), np.float32)
    bcomb[:, 0] = np.concatenate([b1, b1])
    bcomb[:, 1] = np.concatenate([b2, b2])
    bcomb[:, 2] = np.tile(b3, 4)
    bcomb[:, 3] = b4[0]
    bcomb[:, 4] = np.tile(W4[:, 0], 4)

    common = {
        "xT2": xT2,
        "w23": w23,
        "bcomb": bcomb,
    }
    in_maps = []
    for c in range(NCORES):
        xc = x[:, c * RPC : (c + 1) * RPC, :]  # [8, 64, 32]
        wcomb = np.zeros((2 * D, 512), bf16)
        for bp in range(4):
            wcomb[:D, bp * RPC : (bp + 1) * RPC] = xc[2 * bp].T.astype(bf16)
            wcomb[D:, bp * RPC : (bp + 1) * RPC] = xc[2 * bp + 1].T.astype(bf16)
        wcomb[:, 256:384] = w1tbd
        wcomb[:, 384:512] = w1bbd
        in_maps.append({**common, "wcomb": wcomb})
    return in_maps


def assemble(per_core_outs):
    full = np.empty((B, N, N), np.float32)
    for c in range(NCORES):
        full[:, c * RPC : (c + 1) * RPC, :] = per_core_outs[c]
    return full


def run(inputs, trace=False):
    """Returns (full_output, BassKernelResults)."""
    from concourse.bass_utils import run_bass_kernel_spmd

    nc = _get_program()
    in_maps = prep_in_maps(inputs)
    res = run_bass_kernel_spmd(nc, in_maps, list(range(NCORES)), trace=trace)
    full = assemble([res.results[c]["out"] for c in range(NCORES)])
    return full, res


def kernel(**inputs):
    full, _ = run(inputs, trace=False)
    return full



# revision 33
# speedup vs baseline: 1.0188x; 1.0030x over previous
"""Trainium2 Bass kernel for nn_CartesianProductClassifier.

out[b,i,j] = sigmoid(MLP(concat(x[b,j], x[b,i])))  for x [8, 512, 32].

Math restructuring:
  layer1: h1[b,i,j] = relu(x[b,j]@W1[:32] + x[b,i]@W1[32:] + b1)
          = relu(A[b,j] + C[b,i])   with A = x@W1_top (N-sized),
            C = x@W1_bot + b1 (N-sized)  -> first layer is O(N), not O(N^2).
  layers 2-4 run on the N^2 grid with 2 batches packed block-diagonally
  on the 128 partitions; matmuls use f32r (full PE rate for 4-byte data),
  except layer 3 which uses bf16 so its second matmul may write PSUM
  partition base 64 (f32r matmuls must write base 0).  Layer 4 uses
  column-shifted W4 matrices accumulating 64 output rows into one PSUM
  bank so a single sigmoid covers them.

Sharding: core c handles rows i in [64c, 64c+64) of all 8 batches.
"""

import numpy as np

B, N, D = 8, 512, 32
NCORES = 8
RPC = N // NCORES  # rows per core = 64

_PROG = None


def _build_program():
    import concourse.mybir as mybir
    import concourse.tile as tile
    from concourse import bacc
    from concourse.tile import add_dep_helper

    dt = mybir.dt
    F32 = dt.float32
    F32R = dt.float32r
    BF16 = dt.bfloat16
    AF = mybir.ActivationFunctionType
    OP = mybir.AluOpType

    nc = bacc.Bacc(
        "TRN2", target_bir_lowering=False, debug=False, num_devices=NCORES
    )

    # xT2: [64, 4*512] col = bp*512 + j; rows 0:32 features of batch 2bp,
    #      rows 32:64 of batch 2bp+1.   xcT2: same layout, col = bp*64 + i.
    xT2 = nc.dram_tensor("xT2", [2 * D, 4 * N], BF16, kind="ExternalInput")
    # wcomb packs [xcT2 (256) | w1tbd (128) | w1bbd (128)] on 64 partitions
    wcomb = nc.dram_tensor("wcomb", [2 * D, 512], BF16, kind="ExternalInput")
    # w23 packs [w2bd (128) | w3bd (64)] on 128 partitions
    w23 = nc.dram_tensor("w23", [128, 192], BF16, kind="ExternalInput")
    # bcomb packs [b1s | b2s | b3s | b4s | w4c-tiled] columns
    bcomb = nc.dram_tensor("bcomb", [128, 5], F32, kind="ExternalInput")
    out = nc.dram_tensor("out", [B, RPC, N], F32, kind="ExternalOutput")

    with tile.TileContext(nc) as tc:
        with (
            tc.tile_pool(name="const", bufs=1) as const,
            tc.tile_pool(name="h1p", bufs=10) as h1p,
            tc.tile_pool(name="h2p", bufs=8) as h2p,
            tc.tile_pool(name="h3p", bufs=8) as h3p,
            tc.tile_pool(name="sigp", bufs=2) as sigp,
            tc.tile_pool(name="psA", bufs=2, space="PSUM") as psA,
            tc.tile_pool(name="psB", bufs=1, space="PSUM") as psB,
            tc.tile_pool(name="psC", bufs=2, space="PSUM") as psC,
        ):
            # ---------- constant loads ----------
            # each dma_start costs ~600ns of issue time on the sync queue, so
            # order by what gates the pipeline: wcomb (stage A weights + xc),
            # bcomb (biases), xT2, w23
            # dummy ACT op first: forces the activation-table load to happen
            # during the DMA issue phase instead of on the first real copy
            scratch = const.tile([128, 1], F32, tag="scratch")
            nc.vector.memset(scratch[:], 0.0)
            nc.scalar.activation(scratch[:], scratch[:], AF.Relu, scale=1.0)
            # w4sh is built on-chip (mostly zeros): zero it while DMAs fly
            w4sh_sb = const.tile([128, 32 * 128], BF16, tag="w4sh")
            nc.vector.memset(w4sh_sb[:], 0.0)
            # two DMA queues in parallel: sync gets wcomb/w23/xT2-rest,
            # scalar-queue gets the small bcomb + the urgent xT2 bp0 slice
            wcomb_sb = const.tile([2 * D, 512], BF16, tag="wcomb")
            nc.sync.dma_start(wcomb_sb[:], wcomb[:])
            bcomb_sb = const.tile([128, 5], F32, tag="bcomb")
            nc.scalar.dma_start(bcomb_sb[:], bcomb[:])
            xT2_sb = const.tile([2 * D, 4 * N], BF16, tag="xT2")
            nc.scalar.dma_start(xT2_sb[:, 0:N], xT2[:, 0:N])
            w23_sb = const.tile([128, 192], BF16, tag="w23")
            nc.sync.dma_start(w23_sb[:], w23[:])
            nc.sync.dma_start(xT2_sb[:, N:], xT2[:, N:])
            xcT2_sb = wcomb_sb[:, 0:256]
            w1tbd_sb = wcomb_sb[:, 256:384]
            w1bbd_sb = wcomb_sb[:, 384:512]
            w2bd_sb = w23_sb[:, 0:128]
            w3bd_sb = w23_sb[:, 128:192]
            b1s_sb = bcomb_sb[:, 0:1]
            b2s_sb = bcomb_sb[:, 1:2]
            b3s_sb = bcomb_sb[:, 2:3]
            b4s_sb = bcomb_sb[:, 3:4]
            # scatter w4 columns into the zeroed w4sh: block r has w4c at
            # rows 32r:32r+32, cols 64*(r&1) + (r>>1) + 130*t  (t = 0..31)
            for r in range(4):
                base = 64 * (r & 1) + (r >> 1)
                nc.vector.tensor_copy(
                    w4sh_sb[
                        32 * r : 32 * r + 32, base : base + 130 * 31 + 1 : 130
                    ],
                    bcomb_sb[32 * r : 32 * r + 32, 4:5].broadcast_to([32, 32]),
                )

            # ---------- stage A: AT2 [128, 2048], Cpp [128, 256] ----------
            # AT2[:, bp*512 + j]: p<64 -> A-hidden of batch 2bp at col j,
            #                     p>=64 -> batch 2bp+1.
            AT2 = const.tile([128, 4 * N], BF16, tag="AT2")
            Cpp = const.tile([128, 4 * RPC], F32, tag="Cpp")
            # Cpp's matmul first: it only needs wcomb (xT2 still in flight)
            psc = psA.tile([128, N], F32, tag="ps2", name="psc")
            nc.tensor.matmul(
                psc[:, 0 : 4 * RPC],
                w1bbd_sb,
                xcT2_sb,
                start=True,
                stop=True,
            )
            # Cpp = psc + b1 (no relu here; relu happens after adding A)
            nc.vector.tensor_scalar(
                Cpp[:], psc[:, 0 : 4 * RPC], b1s_sb, None, OP.add
            )
            psa0 = psA.tile([128, N], F32, tag="ps2", name="psa0")
            nc.tensor.matmul(
                psa0[:], w1tbd_sb, xT2_sb[:, 0:N], start=True, stop=True
            )
            nc.vector.tensor_copy(AT2[:, 0:N], psa0[:])
            for bp in range(1, 4):
                psa = psA.tile([128, N], F32, tag="ps2", name=f"psa{bp}")
                nc.tensor.matmul(
                    psa[:],
                    w1tbd_sb,
                    xT2_sb[:, bp * N : (bp + 1) * N],
                    start=True,
                    stop=True,
                )
                nc.scalar.activation(
                    AT2[:, bp * N : (bp + 1) * N], psa[:], AF.Copy, scale=1.0
                )

            # ---------- main loop: software-pipelined 2-twin pairs ----------
            # 64 global pairs (bp = pair // 16, t in {2u, 2u+1}).  Per pair
            # cycle we emit: h1 x4 + mm2 x4 (pair p), relu2 x2 (pair p),
            # mm3 x4 + relu3 x2 (pair p-1), mm4 x2 (pair p-2).  The 2-stage
            # skew keeps the PE stream free of same-cycle waits and halves
            # weight-set reloads (w2 once per pair).
            sig_tiles = {}

            def emit_h1(tw):
                bp, t = tw
                out_t = []
                for k, ii in enumerate((2 * t, 2 * t + 1)):
                    h1 = h1p.tile(
                        [128, N], BF16, tag="h1", name=f"h1_{bp}_{t}_{k}"
                    )
                    nc.vector.tensor_scalar(
                        h1[:],
                        AT2[:, bp * N : (bp + 1) * N],
                        Cpp[:, bp * RPC + ii : bp * RPC + ii + 1],
                        0.0,
                        OP.add,
                        OP.max,
                    )
                    out_t.append(h1)
                return out_t

            def emit_mm2(tw, h1s):
                bp, t = tw
                ps2d = psA.tile(
                    [128, 2 * N], F32, tag="ps2", name=f"ps2_{bp}_{t}"
                )
                nc.tensor.matmul(
                    ps2d[:, 0:N], w2bd_sb, h1s[0][:], start=True, stop=True
                )
                nc.tensor.matmul(
                    ps2d[:, N : 2 * N],
                    w2bd_sb,
                    h1s[1][:],
                    start=True,
                    stop=True,
                    skip_group_check=True,
                )
                return ps2d

            def emit_relu2(tw, ps2d):
                bp, t = tw
                h2ab = h2p.tile([128, 2 * N], BF16, tag="h2", name=f"h2_{bp}_{t}")
                nc.scalar.activation(
                    h2ab[:], ps2d[:], AF.Relu, bias=b2s_sb, scale=1.0
                )
                return h2ab

            def emit_mm3_pair(p, tws, h2abs):
                # both twins' mm3 into one 2-bank tile so relu3 is one op
                ps3 = psB.tile([128, 2 * N], F32, tag="ps3", name=f"ps3p_{p}")
                for k, h2ab in enumerate(h2abs):
                    nc.tensor.matmul(
                        ps3[0:64, k * N : (k + 1) * N],
                        w3bd_sb,
                        h2ab[:, 0:N],
                        start=True,
                        stop=True,
                        skip_group_check=True,
                    )
                    nc.tensor.matmul(
                        ps3[64:128, k * N : (k + 1) * N],
                        w3bd_sb,
                        h2ab[:, N : 2 * N],
                        start=True,
                        stop=True,
                        skip_group_check=True,
                    )
                return ps3

            def emit_relu3_pair(p, ps3):
                h3 = h3p.tile([128, 2 * N], BF16, tag="h3", name=f"h3p_{p}")
                # every 22nd pair's relu3 runs on the scalar engine to
                # balance DVE (h1+relu3) against ACT (relu2)
                if p % 22 == 5:
                    nc.scalar.activation(
                        h3[:], ps3[:], AF.Relu, bias=b3s_sb, scale=1.0
                    )
                else:
                    nc.vector.tensor_scalar(
                        h3[:], ps3[:], b3s_sb, 0.0, OP.add, OP.max
                    )
                return h3

            def emit_mm4(tw, h3):
                bp, t = tw
                if bp not in sig_tiles:
                    sig_tiles[bp] = psC.tile(
                        [128, N], F32, tag="sig", name=f"sig{bp}"
                    )
                nc.tensor.matmul(
                    sig_tiles[bp][:],
                    w4sh_sb[:, t * 128 : (t + 1) * 128],
                    h3[:],
                    start=(t == 0),
                    stop=(t == 31),
                    skip_group_check=True,
                )
                if t == 31:
                    sig_ps = sig_tiles.pop(bp)
                    sig_sb = sigp.tile(
                        [128, N], F32, tag="sig_sb", name=f"sigsb{bp}"
                    )
                    # split the sigmoid by column half so the out-DMAs start
                    # earlier; spread DMAs across both queues so the final
                    # drain overlaps
                    for h in range(2):
                        cs = slice(h * (N // 2), (h + 1) * (N // 2))
                        nc.scalar.activation(
                            sig_sb[:, cs],
                            sig_ps[:, cs],
                            AF.Sigmoid,
                            bias=b4s_sb,
                            scale=1.0,
                        )
                        nc.sync.dma_start(out[2 * bp, :, cs], sig_sb[0:64, cs])
                        nc.gpsimd.dma_start(
                            out[2 * bp + 1, :, cs], sig_sb[64:128, cs]
                        )

            NP = 64  # global pairs
            state = {}  # pair -> dict of stage products
            for p in range(NP + 2):
                if p < NP:
                    tws = [(p // 16, 2 * (p % 16)), (p // 16, 2 * (p % 16) + 1)]
                    st = {"tws": tws}
                    st["h1"] = [emit_h1(tw) for tw in tws]
                    st["ps2"] = [
                        emit_mm2(tw, h1s) for tw, h1s in zip(tws, st["h1"])
                    ]
                    st["h2"] = [
                        emit_relu2(tw, ps2d)
                        for tw, ps2d in zip(tws, st["ps2"])
                    ]
                    state[p] = st
                if 0 <= p - 1 < NP:
                    st = state[p - 1]
                    ps3 = emit_mm3_pair(p - 1, st["tws"], st["h2"])
                    h3pair = emit_relu3_pair(p - 1, ps3)
                    st["h3"] = [h3pair[:, 0:N], h3pair[:, N : 2 * N]]
                if 0 <= p - 2 < NP:
                    st = state.pop(p - 2)
                    for tw, h3 in zip(st["tws"], st["h3"]):
                        emit_mm4(tw, h3)

    nc.compile()
    return nc


def _get_program():
    global _PROG
    if _PROG is None:
        _PROG = _build_program()
    return _PROG


def prep_in_maps(inputs):
    import ml_dtypes

    x = np.ascontiguousarray(np.asarray(inputs["x"], dtype=np.float32))
    W1 = np.asarray(inputs["W1"], dtype=np.float32)
    b1 = np.asarray(inputs["b1"], dtype=np.float32)
    W2 = np.asarray(inputs["W2"], dtype=np.float32)
    b2 = np.asarray(inputs["b2"], dtype=np.float32)
    W3 = np.asarray(inputs["W3"], dtype=np.float32)
    b3 = np.asarray(inputs["b3"], dtype=np.float32)
    W4 = np.asarray(inputs["W4"], dtype=np.float32)
    b4 = np.asarray(inputs["b4"], dtype=np.float32)

    bf16 = ml_dtypes.bfloat16
    w1tbd = np.zeros((2 * D, 128), bf16)
    w1tbd[:D, :64] = W1[:D].astype(bf16)
    w1tbd[D:, 64:] = W1[:D].astype(bf16)
    w1bbd = np.zeros((2 * D, 128), bf16)
    w1bbd[:D, :64] = W1[D:].astype(bf16)
    w1bbd[D:, 64:] = W1[D:].astype(bf16)
    w23 = np.zeros((128, 192), bf16)
    w23[:64, 0:64] = W2.astype(bf16)
    w23[64:, 64:128] = W2.astype(bf16)
    w23[:64, 128:160] = W3.astype(bf16)
    w23[64:, 160:192] = W3.astype(bf16)
    # xT2 [64, 4*512]: col = bp*512 + j, rows 0:32 batch 2bp, 32:64 batch 2bp+1
    xT2 = np.zeros((2 * D, 4 * N), bf16)
    for bp in range(4):
        xT2[:D, bp * N : (bp + 1) * N] = x[2 * bp].T.astype(bf16)
        xT2[D:, bp * N : (bp + 1) * N] = x[2 * bp + 1].T.astype(bf16)

    bcomb = np.zeros((128, 5), np.float32)
    bcomb[:, 0] = np.concatenate([b1, b1])
    bcomb[:, 1] = np.concatenate([b2, b2])
    bcomb[:, 2] = np.tile(b3, 4)
    bcomb[:, 3] = b4[0]
    bcomb[:, 4] = np.tile(W4[:, 0], 4)

    common = {
        "xT2": xT2,
        "w23": w23,
        "bcomb": bcomb,
    }
    in_maps = []
    for c in range(NCORES):
        xc = x[:, c * RPC : (c + 1) * RPC, :]  # [8, 64, 32]
        wcomb = np.zeros((2 * D, 512), bf16)
        for bp in range(4):
            wcomb[:D, bp * RPC : (bp + 1) * RPC] = xc[2 * bp].T.astype(bf16)
            wcomb[D:, bp * RPC : (bp + 1) * RPC] = xc[2 * bp + 1].T.astype(bf16)
        wcomb[:, 256:384] = w1tbd
        wcomb[:, 384:512] = w1bbd
        in_maps.append({**common, "wcomb": wcomb})
    return in_maps


def assemble(per_core_outs):
    full = np.empty((B, N, N), np.float32)
    for c in range(NCORES):
        full[:, c * RPC : (c + 1) * RPC, :] = per_core_outs[c]
    return full


def run(inputs, trace=False):
    """Returns (full_output, BassKernelResults)."""
    from concourse.bass_utils import run_bass_kernel_spmd

    nc = _get_program()
    in_maps = prep_in_maps(inputs)
    res = run_bass_kernel_spmd(nc, in_maps, list(range(NCORES)), trace=trace)
    full = assemble([res.results[c]["out"] for c in range(NCORES)])
    return full, res


def kernel(**inputs):
    full, _ = run(inputs, trace=False)
    return full

